# revision 38
# baseline (speedup 1.0000x reference)
"""NeighborhoodAttention1D kernel for 8 Trainium2 NeuronCores.

Sequence-parallel: core = (batch b = core//4, chunk j = core%4), each core
owns 1024 queries with a 16-token halo (TOK=1056 local tokens).

v6 over the 86.9us baseline (trace-driven):
  - Input DMAs consolidated 43 -> 8 and issued from BOTH HWDGE rings
    (sync + scalar): the baseline serialized ~27us of DMA_DIRECT2D issue
    on the sync queue, stretching input load to 30us.
  - Q/K in fp8 DoubleRow; V kept fully bf16 (fp8 on either V operand
    costs ~2e-2 rel err by itself - attention averaging shrinks signal
    and noise alike for zero-mean x, and weight error never averages).
  - qkk padded to 1088 cols + uniform kw=128 tail tiles, so scores/exp/
    mask run over a compact [128, 1024] layout: exp is 3 ACT ops/head,
    mask-mult 2 DVE ops/head.
  - attnV lhsT widened to 128 query-cols (junk tail rows unused) so FWL
    engages on the weight load (~3x faster LDW than 96-col loads).
  - ao transposes as regular matmuls against identity (~2x faster than
    transpose-mode, counts as PE activity for the HAM clock gate).
  - projection bias applied on host (exact f32); no bias matmuls.
  - softmax normalize split ~50/50 between Vector and GpSimd (was 24us
    serial on GpSimd alone); out-proj eviction on ACT; memsets GpSimd.
"""

from contextlib import ExitStack

import ml_dtypes
import numpy as np

import concourse.bass as bass
import concourse.tile as tile
from concourse import bacc, mybir
from concourse.bass_utils import run_bass_kernel_spmd
from concourse.masks import make_identity

B, L, DIM = 2, 4096, 512
HEADS, KS = 8, 33
HD = DIM // HEADS          # 64
SCALE = HD ** -0.5
NCORES = 8
CHUNK = 1024
HALO = KS // 2             # 16
TOK = CHUNK + 2 * HALO     # 1056
TOKP = 1088                # padded key cols (zero tail) for uniform kw=128
NS = 11                    # attention tiles: 10 x 96q + 1 x 64q
QCOLS = 1024               # compact query cols = 10*96 + 64
WS = 32.0                  # fp8 weight scale
MNEG = -50.0               # additive mask: exp(s-50) ~ 2e-22 ~ 0

BF = mybir.dt.bfloat16
F32 = mybir.dt.float32
FP8 = mybir.dt.float8e4
NPBF = ml_dtypes.bfloat16
NP8 = ml_dtypes.float8_e4m3
DR = mybir.MatmulPerfMode.DoubleRow
MUL = mybir.AluOpType.mult
ADD = mybir.AluOpType.add

_cache = {}


def _qn(s):
    return 96 if s < 10 else 64


def _qpieces(s):
    """query pieces of tile s against 128-aligned output tiles:
    (b0_in_tile_s, n, t, f0_in_tile_t), block-legal on BOTH partition
    bases (b0 for the aoun/po side, f0 for the aon128 side)."""
    qn = _qn(s)

    def blk(b):
        return 128 if b == 0 else (64 if b == 64 else 32)

    out = []
    q = 96 * s
    while q < 96 * s + qn:
        t = q // 128
        b0 = q - 96 * s
        f0 = q - 128 * t
        n = min(128 * (t + 1) - q, 96 * s + qn - q, blk(b0), blk(f0))
        out.append((b0, n, t, f0))
        q += n
    return out


def _build_bass(niter=1, parts=(1, 2, 3, 4)):
    nc = bacc.Bacc("TRN2", target_bir_lowering=False, debug=False,
                   num_devices=NCORES)

    x_d = nc.dram_tensor("x_dr", [128, 2, 2, TOK], FP8,
                         kind="ExternalInput").ap()
    wq_d = nc.dram_tensor("wq_dr", [128, 2, 2, 2 * DIM], FP8,
                          kind="ExternalInput").ap()
    xb_d = nc.dram_tensor("x_bf", [128, 4, TOK], BF,
                          kind="ExternalInput").ap()
    wv_d = nc.dram_tensor("wv_bf", [128, 4, DIM], BF,
                          kind="ExternalInput").ap()
    wp_d = nc.dram_tensor("wp_bf", [128, 4, DIM], BF,
                          kind="ExternalInput").ap()
    bqe_d = nc.dram_tensor("bqe", [128, 4], F32, kind="ExternalInput").ap()
    masks_d = nc.dram_tensor("masks", [128, QCOLS], BF,
                             kind="ExternalInput").ap()
    out_d = nc.dram_tensor("out", [CHUNK, DIM], BF, kind="ExternalOutput").ap()
    itercheck = None
    if niter > 1:
        itercheck = nc.dram_tensor("itercheck", [1, 8], F32,
                                   kind="ExternalOutput").ap()

    with tile.TileContext(nc) as tc, ExitStack() as ctx:
        sb = ctx.enter_context(tc.tile_pool(name="sb", bufs=1))
        ps = ctx.enter_context(tc.tile_pool(name="ps", bufs=1, space="PSUM"))
        work = ctx.enter_context(tc.tile_pool(name="work", bufs=1))

        x_sb = sb.tile([128, 2, 2, TOK], FP8, tag="x", name="x")
        wq_sb = sb.tile([128, 2, 2, 2 * DIM], FP8, tag="wq", name="wq")
        xb_sb = sb.tile([128, 4, TOK], BF, tag="xb", name="xb")
        wv_sb = sb.tile([128, 4, DIM], BF, tag="wv", name="wv")
        wp_sb = sb.tile([128, 4, DIM], BF, tag="wp", name="wp")
        bqe_sb = sb.tile([128, 4], F32, tag="bqe", name="bqe")
        masks_sb = sb.tile([128, QCOLS], BF, tag="masks", name="masks")
        ident = sb.tile([128, 128], BF, tag="ident", name="ident")
        wz = sb.tile([128, 128], BF, tag="wz", name="wz")

        qkq = [sb.tile([128, CHUNK], BF, tag=f"qkq{o}", name=f"qkq{o}")
               for o in range(4)]
        qkk = [sb.tile([128, TOKP], BF, tag=f"qkk{o}", name=f"qkk{o}")
               for o in range(4)]
        vn = [sb.tile([128, 8, 65], BF, tag=f"vn{s}", name=f"vn{s}")
              for s in range(NS)]
        mskh = [sb.tile([128, TOKP], BF, tag=f"mskh{h}", name=f"mskh{h}")
                for h in range(HEADS)]
        aoun = [sb.tile([96, 8, 65], BF, tag=f"aoun{s}", name=f"aoun{s}")
                for s in range(NS)]
        aon128 = [sb.tile([128, 512], BF, tag=f"aon128_{t}", name=f"aon{t}")
                  for t in range(8)]

        nc.vector.memset(wz[:], 0.0)
        make_identity(nc, ident[:])
        if itercheck is not None:
            ic_sb = sb.tile([1, 8], F32, tag="ic", name="ic")
            nc.vector.memset(ic_sb[:], float(niter))

        def emit_loads():
            # two HWDGE rings: sync takes the tensors the first matmuls
            # need; scalar takes the rest. ~600ns issue cost per DMA.
            # x split so the first q/k chunk (cols 16:528) starts sooner;
            # masks/wp DMAs are issued later in the stream (their
            # consumers run last) so early HBM bandwidth goes to x/wq/xb.
            nc.sync.dma_start(x_sb[:, :, :, 0:528], x_d[:, :, :, 0:528])
            nc.sync.dma_start(wq_sb[:], wq_d[:])
            nc.sync.dma_start(x_sb[:, :, :, 528:TOK], x_d[:, :, :, 528:TOK])
            nc.sync.dma_start(xb_sb[:], xb_d[:])
            nc.scalar.dma_start(bqe_sb[:], bqe_d[:])
            nc.scalar.dma_start(wv_sb[:], wv_d[:])

        for _it in range(niter):
            if itercheck is not None:
                nc.sync.dma_start(itercheck[:], ic_sb[:])
            emit_loads()
            # PE warmup: ramps the tensor-engine pstate while input DMAs
            # land; runs on a zeroed tile (no data deps beyond the memset)
            wrm = ps.tile([128, 512], F32, tag="big", name="wrm", bufs=2)
            for _w in range(30):
                nc.tensor.matmul(wrm[:, 0:128], lhsT=wz[:], rhs=wz[:],
                                 start=True, stop=True)

            # per-iteration constant regions (GpSimd - idle in phase 1)
            for s in range(NS):
                nc.gpsimd.memset(vn[s][:, :, 64:65], 1.0)
            nc.gpsimd.memset(vn[10][96:128, :, :], 0.0)
            for o in range(4):
                nc.gpsimd.memset(qkk[o][:, TOK:TOKP], 0.0)
            for h in range(HEADS):
                nc.gpsimd.memset(mskh[h][:, QCOLS:TOKP], 0.0)

            # ---- phase 1: q/k (feature-major, fp8 DR) ----
            for pair in (range(4) if 1 in parts else []):
                for o in (pair, 4 + pair):
                    isq = o < 4
                    dst = qkq[o] if isq else qkk[o - 4]
                    toff = HALO if isq else 0
                    wcol = o * 128
                    chunks = ([(0, 512), (512, 512)] if isq
                              else [(0, 512), (512, 512), (1024, 32)])
                    for (t0, tw) in chunks:
                        p = ps.tile([128, 512], F32, tag="big", name="p1",
                                    bufs=2)
                        for g in range(2):
                            nc.tensor.matmul(
                                p[:, :tw],
                                lhsT=wq_sb[:, g, :, wcol:wcol + 128],
                                rhs=x_sb[:, g, :, toff + t0:toff + t0 + tw],
                                start=(g == 0), stop=(g == 1), perf_mode=DR,
                            )
                        if isq:
                            nc.scalar.activation(
                                out=dst[:, t0:t0 + tw], in_=p[:, :tw],
                                func=mybir.ActivationFunctionType.Identity,
                                bias=bqe_sb[:, o:o + 1], scale=SCALE / WS,
                            )
                        else:
                            nc.vector.tensor_scalar_mul(
                                dst[:, t0:t0 + tw], p[:, :tw], 1.0 / WS)
            nc.scalar.dma_start(masks_sb[:], masks_d[:])

            # ---- phase 1: V (token-major, bf16: weight-side OR x-side fp8
            #      both blow the error budget on this path) ----
            for s in (range(NS) if 1 in parts else []):
                kw = 128 if s < 10 else 96
                p = ps.tile([128, 512], F32, tag="big", name="pv", bufs=2)
                for i in range(4):
                    nc.tensor.matmul(
                        p[:kw, :],
                        lhsT=xb_sb[:, i, 96 * s:96 * s + kw],
                        rhs=wv_sb[:, i, :],
                        start=(i == 0), stop=(i == 3),
                    )
                nc.vector.tensor_copy(
                    vn[s][:kw, :, 0:64],
                    p[:kw, :].rearrange("p (h c) -> p h c", h=8))
            nc.scalar.dma_start(wp_sb[:], wp_d[:])

            # ---- phase 2: scores (+additive mask in psum) -> exp ----
            # groups: A = s0-4 (480 cols), B = s5-9 (480), C = s10 (64).
            # The mask is accumulated into the score psum as -50*(1-band)
            # via an identity matmul, so exp emits masked weights directly.
            # start only on the first MM: start=True marks the WHOLE 2KB
            # psum zero-region pending-zero, so a later start would make
            # the mask accumulate overwrite earlier tiles' scores.
            for h in (range(HEADS) if 2 in parts else []):
                hb = (h % 2) * 64
                qT = qkq[h // 2][hb:hb + 64, :]
                kT = qkk[h // 2][hb:hb + 64, :]
                for (tag, s0, nsg, c0) in (("pSA", 0, 5, 0),
                                           ("pSB", 5, 5, 480),
                                           ("pSC", 10, 1, 960)):
                    cols = sum(_qn(s) for s in range(s0, s0 + nsg))
                    pS = ps.tile([128, cols], F32, tag=tag, name=tag, bufs=1)
                    off = 0
                    for s in range(s0, s0 + nsg):
                        qn = _qn(s)
                        nc.tensor.matmul(
                            pS[:, off:off + qn],
                            lhsT=kT[:, 96 * s:96 * s + 128],
                            rhs=qT[:, 96 * s:96 * s + qn],
                            start=(s == s0), stop=False,
                        )
                        off += qn
                    nc.tensor.matmul(
                        pS[:, 0:cols], lhsT=ident[:],
                        rhs=masks_sb[:, c0:c0 + cols],
                        start=False, stop=True,
                    )
                    nc.scalar.activation(
                        out=mskh[h][:, c0:c0 + cols], in_=pS[:, :cols],
                        func=mybir.ActivationFunctionType.Exp)

            # ---- phase 3: attnV + normalize + transpose + proj ----
            t_emitted = 0
            norm_i = 0

            def emit_tblock(t):
                # regular matmul against identity: same transpose, ~2x
                # faster than transpose-mode and counts as PE activity
                # for the HAM clock (psum out must be f32 in this mode)
                pT4 = ps.tile([128, 512], F32, tag="big", name="pT4", bufs=2)
                for c4 in range(4):
                    nc.tensor.matmul(
                        pT4[:, 128 * c4:128 * c4 + 128],
                        lhsT=aon128[t][:, 128 * c4:128 * c4 + 128],
                        rhs=ident[:], start=True, stop=True,
                    )
                aoT_t = work.tile([128, 4, 128], BF, tag="aoT", name="aoT",
                                  bufs=2)
                nc.vector.tensor_copy(
                    aoT_t[:], pT4[:].rearrange("p (a b) -> p a b", a=4))
                pout = ps.tile([128, 512], F32, tag="big", name="pout",
                               bufs=2)
                for i in range(4):
                    nc.tensor.matmul(
                        pout[:], lhsT=aoT_t[:, i, :],
                        rhs=wp_sb[:, i, :], start=(i == 0), stop=(i == 3),
                    )
                osb_t = work.tile([128, 512], BF, tag="osb", name="osb",
                                  bufs=2)
                nc.scalar.copy(osb_t[:], pout[:])
                nc.sync.dma_start(out_d[128 * t:128 * t + 128, :], osb_t[:])

            for s in (range(NS) if 3 in parts else []):
                qn = _qn(s)
                for g in range(2):
                    po4 = ps.tile([128, 512], F32, tag="po4", name="po4",
                                  bufs=3)
                    for hh in range(4):
                        h = 4 * g + hh
                        nc.tensor.matmul(
                            po4[:, 65 * hh:65 * hh + 65],
                            lhsT=mskh[h][:, 96 * s:96 * s + 128],
                            rhs=vn[s][:, h, :],
                            start=True, stop=True,
                        )
                    nc.scalar.copy(
                        aoun[s][0:qn, 4 * g:4 * g + 4, :],
                        po4[0:qn, 0:260].rearrange("p (h c) -> p h c", h=4))
                rec_s = work.tile([96, 8], BF, tag="rec", name="rec", bufs=3)
                with nc.allow_low_precision(reason="softmax recip in bf16"):
                    nc.vector.reciprocal(rec_s[0:qn, :],
                                         aoun[s][0:qn, :, 64:65].squeeze(2))
                for (b0, n, t, f0) in _qpieces(s):
                    eng = nc.vector if norm_i % 2 == 0 else nc.gpsimd
                    norm_i += 1
                    eng.tensor_tensor(
                        aon128[t][f0:f0 + n, :].rearrange(
                            "p (h c) -> p h c", h=8),
                        aoun[s][b0:b0 + n, :, 0:64],
                        rec_s[b0:b0 + n, :].unsqueeze(2).broadcast_to(
                            [n, 8, 64]),
                        op=MUL)
                # emit output tiles whose queries are fully covered
                while (4 in parts and t_emitted < 8
                       and 96 * s + qn >= 128 * (t_emitted + 1)):
                    emit_tblock(t_emitted)
                    t_emitted += 1

        if 4 not in parts:
            dummy = work.tile([128, 512], BF, tag="osb", name="dummy", bufs=2)
            nc.vector.memset(dummy[:], 0.0)
            for t in range(8):
                nc.sync.dma_start(out_d[128 * t:128 * t + 128, :], dummy[:])

    nc.finalize()
    return nc


def _host_prep(x, w_qkv, b_qkv, w_proj, b_proj):
    x = np.asarray(x, np.float32)
    w_qkv = np.asarray(w_qkv, np.float32)
    b_qkv = np.asarray(b_qkv, np.float32)
    w_proj = np.asarray(w_proj, np.float32)
    b_proj = np.asarray(b_proj, np.float32)

    def dr_pack(wT, ncols):
        # feature f = g*256 + s*128 + p  ->  [128, 2, 2, ncols]
        return np.clip(wT.reshape(2, 2, 128, ncols).transpose(2, 0, 1, 3),
                       -448, 448).astype(NP8)

    wq_dr = dr_pack(w_qkv.T[:, :2 * DIM] * WS, 2 * DIM)
    wv_bf = w_qkv.T[:, 2 * DIM:3 * DIM].reshape(4, 128, DIM) \
        .transpose(1, 0, 2).copy().astype(NPBF)
    wp_bf = w_proj.T.reshape(4, 128, DIM).transpose(1, 0, 2).copy() \
        .astype(NPBF)
    bqe = (SCALE * b_qkv[:DIM]).reshape(4, 128).T.copy().astype(np.float32)

    starts = np.clip(np.arange(L) - HALO, 0, L - KS)

    in_maps = []
    for core in range(NCORES):
        b, j = divmod(core, 4)
        base = j * CHUNK - HALO
        lo, hi = max(0, base), min(L, base + TOK)
        xs = np.zeros((TOK, DIM), np.float32)
        xs[lo - base:hi - base] = x[b, lo:hi]
        x_dr = dr_pack(xs.T, TOK)
        x_bf = xs.T.reshape(4, 128, TOK).transpose(1, 0, 2).copy() \
            .astype(NPBF)

        mk = np.zeros((128, QCOLS), np.float32)
        for s in range(NS):
            qn = _qn(s)
            qg = base + HALO + 96 * s + np.arange(qn)
            kg = base + 96 * s + np.arange(128)
            ws_ = starts[qg]
            band = ((kg[:, None] >= ws_[None, :])
                    & (kg[:, None] <= ws_[None, :] + KS - 1)
                    & (kg[:, None] >= 0) & (kg[:, None] < L))
            mk[:, 96 * s:96 * s + qn] = MNEG * (1.0 - band)
        in_maps.append({
            "x_dr": x_dr, "wq_dr": wq_dr, "x_bf": x_bf, "wv_bf": wv_bf,
            "wp_bf": wp_bf, "bqe": bqe,
            "masks": mk.astype(NPBF),
        })
    return in_maps


def kernel(x, w_qkv, b_qkv, w_proj, b_proj):
    if "nc" not in _cache:
        _cache["nc"] = _build_bass()
    nc = _cache["nc"]
    in_maps = _host_prep(x, w_qkv, b_qkv, w_proj, b_proj)
    res = run_bass_kernel_spmd(nc, in_maps, core_ids=list(range(NCORES)))
    b_qkv = np.asarray(b_qkv, np.float32)
    w_proj = np.asarray(w_proj, np.float32)
    bpb = (np.asarray(b_proj, np.float32)
           + b_qkv[2 * DIM:3 * DIM] @ w_proj.T)
    full = np.empty((B, L, DIM), np.float32)
    for core in range(NCORES):
        b, j = divmod(core, 4)
        full[b, j * CHUNK:(j + 1) * CHUNK] = (
            res.results[core]["out"].astype(np.float32) + bpb)
    return full


# revision 41
# speedup vs baseline: 1.0111x; 1.0111x over previous
"""NeighborhoodAttention1D kernel for 8 Trainium2 NeuronCores.

Sequence-parallel: core = (batch b = core//4, chunk j = core%4), each core
owns 1024 queries with a 16-token halo (TOK=1056 local tokens).

v6 over the 86.9us baseline (trace-driven):
  - Input DMAs consolidated 43 -> 8 and issued from BOTH HWDGE rings
    (sync + scalar): the baseline serialized ~27us of DMA_DIRECT2D issue
    on the sync queue, stretching input load to 30us.
  - Q/K in fp8 DoubleRow; V kept fully bf16 (fp8 on either V operand
    costs ~2e-2 rel err by itself - attention averaging shrinks signal
    and noise alike for zero-mean x, and weight error never averages).
  - qkk padded to 1088 cols + uniform kw=128 tail tiles, so scores/exp/
    mask run over a compact [128, 1024] layout: exp is 3 ACT ops/head,
    mask-mult 2 DVE ops/head.
  - attnV lhsT widened to 128 query-cols (junk tail rows unused) so FWL
    engages on the weight load (~3x faster LDW than 96-col loads).
  - ao transposes as regular matmuls against identity (~2x faster than
    transpose-mode, counts as PE activity for the HAM clock gate).
  - projection bias applied on host (exact f32); no bias matmuls.
  - softmax normalize split ~50/50 between Vector and GpSimd (was 24us
    serial on GpSimd alone); out-proj eviction on ACT; memsets GpSimd.
"""

from contextlib import ExitStack

import ml_dtypes
import numpy as np

import concourse.bass as bass
import concourse.tile as tile
from concourse import bacc, mybir
from concourse.bass_utils import run_bass_kernel_spmd
from concourse.masks import make_identity

B, L, DIM = 2, 4096, 512
HEADS, KS = 8, 33
HD = DIM // HEADS          # 64
SCALE = HD ** -0.5
NCORES = 8
CHUNK = 1024
HALO = KS // 2             # 16
TOK = CHUNK + 2 * HALO     # 1056
TOKP = 1088                # padded key cols (zero tail) for uniform kw=128
NS = 11                    # attention tiles: 10 x 96q + 1 x 64q
QCOLS = 1024               # compact query cols = 10*96 + 64
WS = 32.0                  # fp8 weight scale
MNEG = -50.0               # additive mask: exp(s-50) ~ 2e-22 ~ 0

BF = mybir.dt.bfloat16
F32 = mybir.dt.float32
FP8 = mybir.dt.float8e4
NPBF = ml_dtypes.bfloat16
NP8 = ml_dtypes.float8_e4m3
DR = mybir.MatmulPerfMode.DoubleRow
MUL = mybir.AluOpType.mult
ADD = mybir.AluOpType.add

_cache = {}


def _qn(s):
    return 96 if s < 10 else 64


def _qpieces(s):
    """query pieces of tile s against 128-aligned output tiles:
    (b0_in_tile_s, n, t, f0_in_tile_t), block-legal on BOTH partition
    bases (b0 for the aoun/po side, f0 for the aon128 side)."""
    qn = _qn(s)

    def blk(b):
        return 128 if b == 0 else (64 if b == 64 else 32)

    out = []
    q = 96 * s
    while q < 96 * s + qn:
        t = q // 128
        b0 = q - 96 * s
        f0 = q - 128 * t
        n = min(128 * (t + 1) - q, 96 * s + qn - q, blk(b0), blk(f0))
        out.append((b0, n, t, f0))
        q += n
    return out


def _build_bass(niter=1, parts=(1, 2, 3, 4)):
    nc = bacc.Bacc("TRN2", target_bir_lowering=False, debug=False,
                   num_devices=NCORES)

    x_d = nc.dram_tensor("x_dr", [128, 2, 2, TOK], FP8,
                         kind="ExternalInput").ap()
    wq_d = nc.dram_tensor("wq_dr", [128, 2, 2, 2 * DIM], FP8,
                          kind="ExternalInput").ap()
    xb_d = nc.dram_tensor("x_bf", [128, 4, TOK], BF,
                          kind="ExternalInput").ap()
    wv_d = nc.dram_tensor("wv_bf", [128, 4, DIM], BF,
                          kind="ExternalInput").ap()
    wp_d = nc.dram_tensor("wp_bf", [128, 4, DIM], BF,
                          kind="ExternalInput").ap()
    bqe_d = nc.dram_tensor("bqe", [128, 4], F32, kind="ExternalInput").ap()
    masks_d = nc.dram_tensor("masks", [128, QCOLS], BF,
                             kind="ExternalInput").ap()
    out_d = nc.dram_tensor("out", [CHUNK, DIM], BF, kind="ExternalOutput").ap()
    itercheck = None
    if niter > 1:
        itercheck = nc.dram_tensor("itercheck", [1, 8], F32,
                                   kind="ExternalOutput").ap()

    with tile.TileContext(nc) as tc, ExitStack() as ctx:
        sb = ctx.enter_context(tc.tile_pool(name="sb", bufs=1))
        ps = ctx.enter_context(tc.tile_pool(name="ps", bufs=1, space="PSUM"))
        work = ctx.enter_context(tc.tile_pool(name="work", bufs=1))

        x_sb = sb.tile([128, 2, 2, TOK], FP8, tag="x", name="x")
        wq_sb = sb.tile([128, 2, 2, 2 * DIM], FP8, tag="wq", name="wq")
        xb_sb = sb.tile([128, 4, TOK], BF, tag="xb", name="xb")
        wv_sb = sb.tile([128, 4, DIM], BF, tag="wv", name="wv")
        wp_sb = sb.tile([128, 4, DIM], BF, tag="wp", name="wp")
        bqe_sb = sb.tile([128, 4], F32, tag="bqe", name="bqe")
        masks_sb = sb.tile([128, QCOLS], BF, tag="masks", name="masks")
        ident = sb.tile([128, 128], BF, tag="ident", name="ident")
        wz = sb.tile([128, 128], BF, tag="wz", name="wz")

        qkq = [sb.tile([128, CHUNK], BF, tag=f"qkq{o}", name=f"qkq{o}")
               for o in range(4)]
        qkk = [sb.tile([128, TOKP], BF, tag=f"qkk{o}", name=f"qkk{o}")
               for o in range(4)]
        vn = [sb.tile([128, 8, 65], BF, tag=f"vn{s}", name=f"vn{s}")
              for s in range(NS)]
        mskh = [sb.tile([128, TOKP], BF, tag=f"mskh{h}", name=f"mskh{h}")
                for h in range(HEADS)]
        aoun = [sb.tile([96, 8, 65], BF, tag=f"aoun{s}", name=f"aoun{s}")
                for s in range(NS)]
        aon128 = [sb.tile([128, 512], BF, tag=f"aon128_{t}", name=f"aon{t}")
                  for t in range(8)]

        nc.vector.memset(wz[:], 0.0)
        make_identity(nc, ident[:])
        if itercheck is not None:
            ic_sb = sb.tile([1, 8], F32, tag="ic", name="ic")
            nc.vector.memset(ic_sb[:], float(niter))

        def emit_loads():
            # two HWDGE rings: sync takes the tensors the first matmuls
            # need; scalar takes the rest. ~600ns issue cost per DMA.
            nc.sync.dma_start(x_sb[:], x_d[:])
            nc.sync.dma_start(wq_sb[:], wq_d[:])
            nc.sync.dma_start(xb_sb[:], xb_d[:])
            nc.scalar.dma_start(bqe_sb[:], bqe_d[:])
            nc.scalar.dma_start(wv_sb[:], wv_d[:])
            nc.scalar.dma_start(masks_sb[:], masks_d[:])
            nc.scalar.dma_start(wp_sb[:], wp_d[:])

        for _it in range(niter):
            if itercheck is not None:
                nc.sync.dma_start(itercheck[:], ic_sb[:])
            emit_loads()
            # PE warmup: ramps the tensor-engine pstate while input DMAs
            # land; runs on a zeroed tile (no data deps beyond the memset)
            wrm = ps.tile([128, 512], F32, tag="big", name="wrm", bufs=2)
            for _w in range(30):
                nc.tensor.matmul(wrm[:, 0:128], lhsT=wz[:], rhs=wz[:],
                                 start=True, stop=True)

            # per-iteration constant regions (GpSimd - idle in phase 1)
            for s in range(NS):
                nc.gpsimd.memset(vn[s][:, :, 64:65], 1.0)
            nc.gpsimd.memset(vn[10][96:128, :, :], 0.0)
            for o in range(4):
                nc.gpsimd.memset(qkk[o][:, TOK:TOKP], 0.0)
            for h in range(HEADS):
                nc.gpsimd.memset(mskh[h][:, QCOLS:TOKP], 0.0)

            # ---- phase 1: q/k (feature-major, fp8 DR) ----
            for pair in (range(4) if 1 in parts else []):
                for o in (pair, 4 + pair):
                    isq = o < 4
                    dst = qkq[o] if isq else qkk[o - 4]
                    toff = HALO if isq else 0
                    wcol = o * 128
                    chunks = ([(0, 512), (512, 512)] if isq
                              else [(0, 512), (512, 512), (1024, 32)])
                    for (t0, tw) in chunks:
                        p = ps.tile([128, 512], F32, tag="big", name="p1",
                                    bufs=2)
                        for g in range(2):
                            nc.tensor.matmul(
                                p[:, :tw],
                                lhsT=wq_sb[:, g, :, wcol:wcol + 128],
                                rhs=x_sb[:, g, :, toff + t0:toff + t0 + tw],
                                start=(g == 0), stop=(g == 1), perf_mode=DR,
                            )
                        if isq:
                            nc.scalar.activation(
                                out=dst[:, t0:t0 + tw], in_=p[:, :tw],
                                func=mybir.ActivationFunctionType.Identity,
                                bias=bqe_sb[:, o:o + 1], scale=SCALE / WS,
                            )
                        else:
                            nc.vector.tensor_scalar_mul(
                                dst[:, t0:t0 + tw], p[:, :tw], 1.0 / WS)

            # ---- phase 1: V (token-major, bf16: weight-side OR x-side fp8
            #      both blow the error budget on this path) ----
            for s in (range(NS) if 1 in parts else []):
                kw = 128 if s < 10 else 96
                p = ps.tile([128, 512], F32, tag="big", name="pv", bufs=2)
                for i in range(4):
                    nc.tensor.matmul(
                        p[:kw, :],
                        lhsT=xb_sb[:, i, 96 * s:96 * s + kw],
                        rhs=wv_sb[:, i, :],
                        start=(i == 0), stop=(i == 3),
                    )
                nc.vector.tensor_copy(
                    vn[s][:kw, :, 0:64],
                    p[:kw, :].rearrange("p (h c) -> p h c", h=8))

            # ---- phase 2: scores (+additive mask in psum) -> exp ----
            # groups: A = s0-4 (480 cols), B = s5-9 (480), C = s10 (64).
            # The mask is accumulated into the score psum as -50*(1-band)
            # via an identity matmul, so exp emits masked weights directly.
            # start only on the first MM: start=True marks the WHOLE 2KB
            # psum zero-region pending-zero, so a later start would make
            # the mask accumulate overwrite earlier tiles' scores.
            for h in (range(HEADS) if 2 in parts else []):
                hb = (h % 2) * 64
                qT = qkq[h // 2][hb:hb + 64, :]
                kT = qkk[h // 2][hb:hb + 64, :]
                for (tag, s0, nsg, c0) in (("pSA", 0, 5, 0),
                                           ("pSB", 5, 5, 480),
                                           ("pSC", 10, 1, 960)):
                    cols = sum(_qn(s) for s in range(s0, s0 + nsg))
                    pS = ps.tile([128, cols], F32, tag=tag, name=tag, bufs=1)
                    off = 0
                    for s in range(s0, s0 + nsg):
                        qn = _qn(s)
                        nc.tensor.matmul(
                            pS[:, off:off + qn],
                            lhsT=kT[:, 96 * s:96 * s + 128],
                            rhs=qT[:, 96 * s:96 * s + qn],
                            start=(s == s0), stop=False,
                        )
                        off += qn
                    nc.tensor.matmul(
                        pS[:, 0:cols], lhsT=ident[:],
                        rhs=masks_sb[:, c0:c0 + cols],
                        start=False, stop=True,
                    )
                    nc.scalar.activation(
                        out=mskh[h][:, c0:c0 + cols], in_=pS[:, :cols],
                        func=mybir.ActivationFunctionType.Exp)

            # ---- phase 3: attnV + normalize + transpose + proj ----
            t_emitted = 0
            norm_i = 0

            def emit_tblock(t):
                # regular matmul against identity: same transpose, ~2x
                # faster than transpose-mode and counts as PE activity
                # for the HAM clock (psum out must be f32 in this mode)
                pT4 = ps.tile([128, 512], F32, tag="big", name="pT4", bufs=2)
                for c4 in range(4):
                    nc.tensor.matmul(
                        pT4[:, 128 * c4:128 * c4 + 128],
                        lhsT=aon128[t][:, 128 * c4:128 * c4 + 128],
                        rhs=ident[:], start=True, stop=True,
                    )
                aoT_t = work.tile([128, 4, 128], BF, tag="aoT", name="aoT",
                                  bufs=2)
                nc.vector.tensor_copy(
                    aoT_t[:], pT4[:].rearrange("p (a b) -> p a b", a=4))
                pout = ps.tile([128, 512], F32, tag="big", name="pout",
                               bufs=2)
                for i in range(4):
                    nc.tensor.matmul(
                        pout[:], lhsT=aoT_t[:, i, :],
                        rhs=wp_sb[:, i, :], start=(i == 0), stop=(i == 3),
                    )
                osb_t = work.tile([128, 512], BF, tag="osb", name="osb",
                                  bufs=2)
                nc.scalar.copy(osb_t[:], pout[:])
                nc.sync.dma_start(out_d[128 * t:128 * t + 128, :], osb_t[:])

            for s in (range(NS) if 3 in parts else []):
                qn = _qn(s)
                for g in range(2):
                    po4 = ps.tile([128, 512], F32, tag="po4", name="po4",
                                  bufs=3)
                    for hh in range(4):
                        h = 4 * g + hh
                        nc.tensor.matmul(
                            po4[:, 65 * hh:65 * hh + 65],
                            lhsT=mskh[h][:, 96 * s:96 * s + 128],
                            rhs=vn[s][:, h, :],
                            start=True, stop=True,
                        )
                    nc.scalar.copy(
                        aoun[s][0:qn, 4 * g:4 * g + 4, :],
                        po4[0:qn, 0:260].rearrange("p (h c) -> p h c", h=4))
                rec_s = work.tile([96, 8], BF, tag="rec", name="rec", bufs=3)
                with nc.allow_low_precision(reason="softmax recip in bf16"):
                    nc.vector.reciprocal(rec_s[0:qn, :],
                                         aoun[s][0:qn, :, 64:65].squeeze(2))
                for (b0, n, t, f0) in _qpieces(s):
                    eng = nc.vector if norm_i % 2 == 0 else nc.gpsimd
                    norm_i += 1
                    eng.tensor_tensor(
                        aon128[t][f0:f0 + n, :].rearrange(
                            "p (h c) -> p h c", h=8),
                        aoun[s][b0:b0 + n, :, 0:64],
                        rec_s[b0:b0 + n, :].unsqueeze(2).broadcast_to(
                            [n, 8, 64]),
                        op=MUL)
                # emit output tiles whose queries are fully covered
                while (4 in parts and t_emitted < 8
                       and 96 * s + qn >= 128 * (t_emitted + 1)):
                    emit_tblock(t_emitted)
                    t_emitted += 1

        if 4 not in parts:
            dummy = work.tile([128, 512], BF, tag="osb", name="dummy", bufs=2)
            nc.vector.memset(dummy[:], 0.0)
            for t in range(8):
                nc.sync.dma_start(out_d[128 * t:128 * t + 128, :], dummy[:])

    nc.finalize()
    return nc


def _host_prep(x, w_qkv, b_qkv, w_proj, b_proj):
    x = np.asarray(x, np.float32)
    w_qkv = np.asarray(w_qkv, np.float32)
    b_qkv = np.asarray(b_qkv, np.float32)
    w_proj = np.asarray(w_proj, np.float32)
    b_proj = np.asarray(b_proj, np.float32)

    def dr_pack(wT, ncols):
        # feature f = g*256 + s*128 + p  ->  [128, 2, 2, ncols]
        return np.clip(wT.reshape(2, 2, 128, ncols).transpose(2, 0, 1, 3),
                       -448, 448).astype(NP8)

    wq_dr = dr_pack(w_qkv.T[:, :2 * DIM] * WS, 2 * DIM)
    wv_bf = w_qkv.T[:, 2 * DIM:3 * DIM].reshape(4, 128, DIM) \
        .transpose(1, 0, 2).copy().astype(NPBF)
    wp_bf = w_proj.T.reshape(4, 128, DIM).transpose(1, 0, 2).copy() \
        .astype(NPBF)
    bqe = (SCALE * b_qkv[:DIM]).reshape(4, 128).T.copy().astype(np.float32)

    starts = np.clip(np.arange(L) - HALO, 0, L - KS)

    in_maps = []
    for core in range(NCORES):
        b, j = divmod(core, 4)
        base = j * CHUNK - HALO
        lo, hi = max(0, base), min(L, base + TOK)
        xs = np.zeros((TOK, DIM), np.float32)
        xs[lo - base:hi - base] = x[b, lo:hi]
        x_dr = dr_pack(xs.T, TOK)
        x_bf = xs.T.reshape(4, 128, TOK).transpose(1, 0, 2).copy() \
            .astype(NPBF)

        mk = np.zeros((128, QCOLS), np.float32)
        for s in range(NS):
            qn = _qn(s)
            qg = base + HALO + 96 * s + np.arange(qn)
            kg = base + 96 * s + np.arange(128)
            ws_ = starts[qg]
            band = ((kg[:, None] >= ws_[None, :])
                    & (kg[:, None] <= ws_[None, :] + KS - 1)
                    & (kg[:, None] >= 0) & (kg[:, None] < L))
            mk[:, 96 * s:96 * s + qn] = MNEG * (1.0 - band)
        in_maps.append({
            "x_dr": x_dr, "wq_dr": wq_dr, "x_bf": x_bf, "wv_bf": wv_bf,
            "wp_bf": wp_bf, "bqe": bqe,
            "masks": mk.astype(NPBF),
        })
    return in_maps


def kernel(x, w_qkv, b_qkv, w_proj, b_proj):
    if "nc" not in _cache:
        _cache["nc"] = _build_bass()
    nc = _cache["nc"]
    in_maps = _host_prep(x, w_qkv, b_qkv, w_proj, b_proj)
    res = run_bass_kernel_spmd(nc, in_maps, core_ids=list(range(NCORES)))
    b_qkv = np.asarray(b_qkv, np.float32)
    w_proj = np.asarray(w_proj, np.float32)
    bpb = (np.asarray(b_proj, np.float32)
           + b_qkv[2 * DIM:3 * DIM] @ w_proj.T)
    full = np.empty((B, L, DIM), np.float32)
    for core in range(NCORES):
        b, j = divmod(core, 4)
        full[b, j * CHUNK:(j + 1) * CHUNK] = (
            res.results[core]["out"].astype(np.float32) + bpb)
    return full
